# revision 1
# baseline (speedup 1.0000x reference)
"""Patch-entropy (histogram binning) Trainium2 Bass kernel.

Computes, per image: 16x16-patch grayscale 32-bin histogram Shannon entropy,
min/max-normalized per image.  Input x:[64,3,512,512] f32 -> out:[64,32,32] f32.

Sharding: data-parallel over 8 NeuronCores, 8 images per core.

Per-core pipeline (images processed in 2 groups of 4):
  1. DMA rows in channel tiles  [128=(img4,patchrow32), 512] f32.
  2. PE diagonal-weight matmuls accumulate 0.299R+0.587G+0.114B in PSUM
     (bit-identical fp32 add order to the reference).
  3. DVE eviction fuses y = min(32*gray, 31.99...) and a free-dim shuffle to
     (patchcol, row, col) ordering.
  4. nq = -trunc(y) via one fused scalar_tensor_tensor: (y mod 1) - y, exact.
  5. SBUF->SBUF DMA gathers each patch's 256 pixels onto one partition
     (512-byte chunks).
  6. Histogram: per 128-patch tile, per bin b: tensor_scalar is_equal(-b) with
     fused free-dim sum into counts[:, t*32+b]  (bf16, 4x mode).
  7. Entropy: ln via ScalarE, fused multiply+sum per tile, then per-image
     min/max (gpsimd partition all-reduce) normalization.
"""

import numpy as np
from contextlib import ExitStack

import concourse.bass as bass
import concourse.bacc as bacc
import concourse.tile as tile
import concourse.mybir as mybir
from concourse import bass_isa

F32 = mybir.dt.float32
BF16 = mybir.dt.bfloat16
I32 = mybir.dt.int32
AO = mybir.AluOpType
AF = mybir.ActivationFunctionType

N_CORES = 8
IMG_PER_CORE = 8
C, H, W = 3, 512, 512
PS = 16            # patch size
NB = 32            # histogram bins
PH = PW = 32       # patches per image side
GRAY_W = (0.299, 0.587, 0.114)
EPS = 1e-8
# largest f32 strictly below 32.0 -> trunc gives 31 for clipped pixels
Y_MAX = float(np.nextafter(np.float32(32.0), np.float32(0.0)))


def _build_body(ctx, tc, x, consts, out, n_img):
    nc = tc.nc
    n_grp = n_img // 4
    n_tiles = n_img * 8          # 128-patch tiles per core

    const_pool = ctx.enter_context(tc.tile_pool(name="const", bufs=1))
    ch_pool = ctx.enter_context(tc.tile_pool(name="ch", bufs=3))
    u_pool = ctx.enter_context(tc.tile_pool(name="u", bufs=2))
    psum_misc = ctx.enter_context(tc.tile_pool(name="psum_misc", bufs=1, space="PSUM"))
    y_pool = ctx.enter_context(tc.tile_pool(name="y", bufs=1))
    q_pool = ctx.enter_context(tc.tile_pool(name="q", bufs=2))
    qp_pool = ctx.enter_context(tc.tile_pool(name="qp", bufs=1))
    eq_pool = ctx.enter_context(tc.tile_pool(name="eq", bufs=4))
    ent_pool = ctx.enter_context(tc.tile_pool(name="ent", bufs=1))

    # --- constants arrive via DMA (keeps matmul wait-counts <= 2) ---
    cmat = const_pool.tile([128, 128], F32, tag="cmat")
    nc.sync.dma_start(out=cmat[:], in_=consts[:, 384:512])
    ident = cmat[:, 0:128]
    ones_row = const_pool.tile([1, 128], F32, tag="ones_row")
    nc.vector.memset(ones_row, 1.0)

    counts = ent_pool.tile([128, n_tiles * NB], F32, tag="counts")
    stair = ent_pool.tile([128, n_tiles * 31], F32, tag="stair")

    x_r = x.rearrange("b c (pr r) w -> b c pr r w", r=PS)  # [n_img,3,32,16,512]

    ONE_LT1 = float(np.nextafter(np.float32(1.0), np.float32(0.0)))
    for g in range(n_grp):
        y = y_pool.tile([128, 8192], F32, tag="y")  # noqa
        for r in range(PS):
            xts = []
            for c in range(3):
                xt = ch_pool.tile([128, 512], F32, tag=f"xt{c}")
                nc.sync.dma_start(out=xt[:], in_=x_r[4 * g : 4 * g + 4, c, :, r, :])
                xts.append(xt)
            # gray with the reference's exact fp32 rounding order:
            # ((0.299R) + (0.587G)) + (0.114B), then y = min(g,1-ulp)*32
            t0 = u_pool.tile([128, 512], F32, tag="t0")
            nc.vector.tensor_scalar(t0, xts[0], GRAY_W[0], None,
                                    op0=AO.mult, op1=AO.bypass)
            t1 = u_pool.tile([128, 512], F32, tag="t1")
            nc.vector.tensor_scalar(t1, xts[1], GRAY_W[1], None,
                                    op0=AO.mult, op1=AO.bypass)
            t2 = u_pool.tile([128, 512], F32, tag="t2")
            nc.vector.tensor_scalar(t2, xts[2], GRAY_W[2], None,
                                    op0=AO.mult, op1=AO.bypass)
            u2 = u_pool.tile([128, 512], F32, tag="u2")
            nc.vector.tensor_add(u2, t0, t1)
            u3 = u_pool.tile([128, 512], F32, tag="u3")
            nc.vector.tensor_add(u3, u2, t2)
            nc.vector.tensor_scalar(
                y.rearrange("p (pc r c) -> p pc r c", pc=32, r=PS, c=PS)[:, :, r, :],
                u3.rearrange("p (pc c) -> p pc c", c=PS),
                ONE_LT1,
                32.0,
                op0=AO.min,
                op1=AO.mult,
            )
        # gather each patch's 256 pixels onto one partition (1KB chunks)
        yp = qp_pool.tile([128, 8192], F32, tag="yp")
        for i_ in range(4):
            for h_ in range(8):
                t_ = i_ * 8 + h_
                nc.sync.dma_start(
                    out=yp[:, 256 * t_ : 256 * (t_ + 1)],
                    in_=y[32 * i_ + 4 * h_ : 32 * i_ + 4 * h_ + 4, :],
                )

        # histogram staircase: S_t = #{y >= t}, t = 1..31 (no truncation
        # needed: trunc(y) >= t <=> y >= t for integer t, y >= 0)
        for t in range(32):
            tg = g * 32 + t
            yslice = yp[:, 256 * t : 256 * (t + 1)]
            for b in range(1, NB):
                eqm = eq_pool.tile([128, 256], BF16, tag="eqm")
                nc.vector.tensor_scalar(
                    eqm,
                    yslice,
                    float(b),
                    None,
                    op0=AO.is_ge,
                    op1=AO.add,
                    accum_out=stair[:, tg * 31 + (b - 1) : tg * 31 + b],
                )

    # counts from staircase differences: c_0 = 256-S_1, c_b = S_b - S_{b+1},
    # c_31 = S_31
    sv = stair.rearrange("p (t b) -> p t b", b=31)
    cv = counts.rearrange("p (t b) -> p t b", b=NB)
    nc.vector.tensor_scalar(cv[:, :, 0], sv[:, :, 0], -1.0, 256.0,
                            op0=AO.mult, op1=AO.add)
    nc.vector.tensor_sub(cv[:, :, 1:31], sv[:, :, 0:30], sv[:, :, 1:31])
    nc.vector.tensor_copy(cv[:, :, 31], sv[:, :, 30])

    # --- entropy ---
    nf = n_tiles * NB
    pe = ent_pool.tile([128, nf], F32, tag="pe")
    nc.vector.tensor_scalar(pe, counts, 1.0 / 256.0, EPS, op0=AO.mult, op1=AO.add)
    lnpe = ent_pool.tile([128, nf], F32, tag="lnpe")
    nc.scalar.activation(lnpe, pe, AF.Ln)
    entS = ent_pool.tile([128, n_tiles], F32, tag="entS")
    for t in range(n_tiles):
        dummy = eq_pool.tile([128, NB], F32, tag="entdummy")
        nc.vector.scalar_tensor_tensor(
            dummy,
            counts[:, t * NB : (t + 1) * NB],
            1.0,
            lnpe[:, t * NB : (t + 1) * NB],
            op0=AO.mult,
            op1=AO.mult,
            accum_out=entS[:, t : t + 1],
        )
    ent = ent_pool.tile([128, n_tiles], F32, tag="ent")
    nc.vector.tensor_scalar(ent, entS, -1.0 / 256.0, None, op0=AO.mult, op1=AO.bypass)

    # --- per-image min/max via PE transpose + free-dim reduces ---
    entT_ps = psum_misc.tile([n_tiles, 128], F32, tag="tps_big")
    nc.tensor.transpose(entT_ps, ent, ident)
    entT = ent_pool.tile([n_tiles, 128], F32, tag="entT")
    nc.vector.tensor_copy(entT, entT_ps)
    # per-tile-row max/min over the 128 patches -> two [n_tiles, 1] tiles
    mm_max = ent_pool.tile([n_tiles, 1], F32, tag="mm_max")
    nc.vector.tensor_reduce(mm_max, entT, axis=mybir.AxisListType.X, op=AO.max)
    mm_min = ent_pool.tile([n_tiles, 1], F32, tag="mm_min")
    nc.vector.tensor_reduce(mm_min, entT, axis=mybir.AxisListType.X, op=AO.min)
    mmT_max_ps = psum_misc.tile([1, n_tiles], F32, tag="mmT_max_ps")
    nc.tensor.transpose(mmT_max_ps, mm_max, ident[:n_tiles, :n_tiles])
    mmT_min_ps = psum_misc.tile([1, n_tiles], F32, tag="mmT_min_ps")
    nc.tensor.transpose(mmT_min_ps, mm_min, ident[:n_tiles, :n_tiles])
    mmT_max = ent_pool.tile([1, n_tiles], F32, tag="mmT_max")
    nc.vector.tensor_copy(mmT_max, mmT_max_ps)
    mmT_min = ent_pool.tile([1, n_tiles], F32, tag="mmT_min")
    nc.vector.tensor_copy(mmT_min, mmT_min_ps)
    # per-image max/min
    mimg_max = ent_pool.tile([1, n_img], F32, tag="mimg_max")
    nc.vector.tensor_reduce(
        mimg_max, mmT_max.rearrange("p (i t) -> p i t", t=8),
        axis=mybir.AxisListType.X, op=AO.max,
    )
    mimg_min = ent_pool.tile([1, n_img], F32, tag="mimg_min")
    nc.vector.tensor_reduce(
        mimg_min, mmT_min.rearrange("p (i t) -> p i t", t=8),
        axis=mybir.AxisListType.X, op=AO.min,
    )
    # broadcast each row to all 128 partitions via K=1 matmul
    rmax_ps = psum_misc.tile([128, n_img], F32, tag="bcast")
    nc.tensor.matmul(rmax_ps, ones_row, mimg_max)
    rmin_ps = psum_misc.tile([128, n_img], F32, tag="bcast")
    nc.tensor.matmul(rmin_ps, ones_row, mimg_min)
    rmax = ent_pool.tile([128, n_img], F32, tag="rmax")
    nc.vector.tensor_copy(rmax, rmax_ps)
    rmin = ent_pool.tile([128, n_img], F32, tag="rmin")
    nc.vector.tensor_copy(rmin, rmin_ps)

    denom = ent_pool.tile([128, n_img], F32, tag="denom")
    nc.vector.tensor_sub(denom, rmax, rmin)
    nc.vector.tensor_scalar(denom, denom, EPS, None, op0=AO.add, op1=AO.bypass)
    rden = ent_pool.tile([128, n_img], F32, tag="rden")
    nc.vector.reciprocal(rden, denom)

    norm = ent_pool.tile([128, n_tiles], F32, tag="norm")
    for i in range(n_img):
        nc.vector.tensor_scalar(
            norm[:, 8 * i : 8 * i + 8],
            ent[:, 8 * i : 8 * i + 8],
            rmin[:, i : i + 1],
            rden[:, i : i + 1],
            op0=AO.subtract,
            op1=AO.mult,
        )

    # --- output: PE transpose [128,T] -> [T,128] then contiguous store ---
    normT = psum_misc.tile([n_tiles, 128], F32, tag="tps_big")
    nc.tensor.transpose(normT, norm, ident)
    normT_sb = ent_pool.tile([n_tiles, 128], F32, tag="normT_sb")
    nc.vector.tensor_copy(normT_sb, normT)
    nc.sync.dma_start(
        out=out.rearrange("b (hh hl) pw -> (b hh) (hl pw)", hl=4),
        in_=normT_sb,
    )


def consts_np():
    a = np.zeros((128, 512), np.float32)
    eye = np.eye(128, dtype=np.float32)
    for c in range(3):
        a[:, 128 * c : 128 * (c + 1)] = eye * np.float32(GRAY_W[c])
    a[:, 384:512] = eye
    return a


def build_program(n_img=IMG_PER_CORE):
    nc = bacc.Bacc(target_bir_lowering=True)
    x = nc.declare_dram_parameter("x", [n_img, C, H, W], F32, isOutput=False)
    consts = nc.declare_dram_parameter("consts", [128, 512], F32, isOutput=False)
    out = nc.declare_dram_parameter("out", [n_img, PH, PW], F32, isOutput=True)
    with tile.TileContext(nc) as tc:
        with ExitStack() as ctx:
            _build_body(ctx, tc, x[:], consts[:], out[:], n_img)
    return nc


_CACHED = {}


def _get_program(n_img):
    if n_img not in _CACHED:
        nc = build_program(n_img)
        nc.finalize()
        _CACHED[n_img] = nc
    return _CACHED[n_img]


def kernel(x, patch_size, num_bins):
    assert int(patch_size) == PS and int(num_bins) == NB
    x = np.asarray(x, dtype=np.float32)
    B = x.shape[0]
    assert x.shape == (B, C, H, W) and B % N_CORES == 0
    per = B // N_CORES
    nc = _get_program(per)

    cns = consts_np()
    in_maps = [
        {"x": x[i * per : (i + 1) * per], "consts": cns} for i in range(N_CORES)
    ]
    try:
        from concourse.bass_utils import run_bass_kernel_spmd

        res = run_bass_kernel_spmd(nc, in_maps, list(range(N_CORES)), trace=False)
        return np.concatenate(
            [res.results[i]["out"] for i in range(N_CORES)], axis=0
        )
    except Exception:
        # fallback: cycle-accurate simulator (correct, slow)
        from concourse.bass_interp import CoreSim

        outs = []
        for m in in_maps:
            sim = CoreSim(nc)
            for k, v in m.items():
                sim.tensor(k)[:] = v
            sim.simulate()
            outs.append(np.array(sim.tensor("out")))
        return np.concatenate(outs, axis=0)


if __name__ == "__main__":
    from reference import setup_inputs, reference

    inputs = {k: np.asarray(v) for k, v in setup_inputs().items()}
    expected = np.asarray(reference(**inputs))
    actual = kernel(**inputs)
    err = np.max(np.abs(actual - expected)) / max(1e-12, np.max(np.abs(expected)))
    print("Relative error:", err)

